# revision 28
# baseline (speedup 1.0000x reference)
"""Trainium2 Bass kernel for nn_CosmosPatcher3d.

Computes the Cosmos 3D Haar wavelet patcher: input [1,3,33,704,704] fp32,
temporal causal pad (first frame repeated 4x -> 36 frames), then two full
3D Haar DWT levels. Equivalent to a separable +-1 Hadamard transform over
4x4x4 blocks scaled by 1/64, producing [1,192,9,176,176] fp32 with channel
layout ch = 48*TH2 + 24*W2 + 6*TH1 + 3*W1 + c (TH = 2T+H).

v4 strategy (8 NeuronCores, shard along H: 704 = 8*88):
- t-block 0 is the repeated first frame: its T-highpass bands are exactly
  zero and the rest is a 2D transform of frame 0 -> computed on HOST in
  f32. The device handles t-blocks 1..8 (frames 1..32) only.
- The ENTIRE 3-level butterfly runs inside ONE plain bf16 matmul per
  (chunk, c): both W parities (p1, p2) are moved into the contraction
  dim: partition k = (hh in 8, dt in 4, p2, p1) = 128, output rows
  m = (w1, w2, th2, y', th1) = 128. Each of the 64 block inputs maps to
  exactly one k; cols = x'' (176). PE cost = 1 col/cycle at the 1.2GHz
  mid p-state = 4.84us/t. Ldweights between back-to-back same-weight
  matmuls are free; the 2.4GHz boost resets on sub-us idle gaps and is
  unreachable in steady state, so the schedule assumes 1.2GHz.
- Input bf16 (absmax rel err 1.8e-3), output = FINAL subband values in
  uint8: the x128 scale is folded into the sign matrix (entries +-2,
  exact) and the copies add +128.5 so the truncating f32->uint8 store
  becomes round-to-nearest (q = v*128 + 128, err 3.9e-3; host decodes
  (q-128)/128). Total err ~7e-3 << 2e-2.
- Per-core DMA 17.84MB (11.89 bf16 in + 5.95 uint8 out). Loads: ONE
  whole-t [128, 11616B] HWDGE DMA on sync (11616B descriptors ~427GB/s
  on the serialized 16-engine pool). Stores: one [128, <=2112B] SWDGE
  DMA per 4-chunk group on GPSIMD -- stores must stay OFF the scalar/
  vector queues: a DMA instruction between copies stalls the copy stream
  past the 16-slot PSUM WAR window and the matmuls then wait on copy
  semaphores. Copies alternate scalar/vector (round-robin per matmul).
"""

import ml_dtypes
import numpy as np

import concourse.bacc as bacc
import concourse.mybir as mybir
import concourse.tile as tile
from concourse.bass_utils import run_bass_kernel_spmd

N_CORES = 8
C = 3              # input channels
T_IN = 33          # input frames
H_IN = 704         # input height (global)
W_IN = 704         # input width
H_SH = H_IN // N_CORES       # 88 input rows per core
T_DEV = 8          # device t-blocks (1..8); t=0 on host
X = W_IN // 4      # 176 output cols
Y_SH = H_SH // 4   # 22 output rows per core
NCH = 11           # chunks of nh=8 h-rows per t
GROUPS = [(0, 4), (4, 4), (8, 3)]  # (first chunk, n chunks) per store group
CX = C * X         # 528 free elements per chunk row

_F32 = mybir.dt.float32
_BF16 = mybir.dt.bfloat16
_U8 = mybir.dt.uint8
_BF16_NP = ml_dtypes.bfloat16


def _sgn1d(pos, b2, b1):
    """Composite 2-level Haar sign for position pos in 0..3 (+-1)."""
    s1 = 1.0 if b1 == 0 else (1.0 - 2.0 * (pos % 2))
    s2 = 1.0 if b2 == 0 else (1.0 - 2.0 * (pos // 2))
    return s1 * s2


def _build_w():
    """[128, 128] bf16 sign matrix, all 3 butterfly levels + x2 scale.

    k = hh*16 + dt*4 + p2*2 + p1 (hh in 0..8),
    m = w1*64 + w2*32 + th2*8 + y'*4 + th1 (y' = hh//4).
    Entries +-2 = 128/64: folds the global 1/64 and the x128 uint8 scale.
    """
    w = np.zeros((128, 128), dtype=np.float32)
    for hh in range(8):
        yp, hp = hh // 4, hh % 4
        for dt in range(4):
            for p2 in range(2):
                for p1 in range(2):
                    k = hh * 16 + dt * 4 + p2 * 2 + p1
                    for w1 in range(2):
                        sw1 = 1.0 if w1 == 0 else (1.0 - 2.0 * p1)
                        for w2 in range(2):
                            sw2 = 1.0 if w2 == 0 else (1.0 - 2.0 * p2)
                            for t2 in range(2):
                                for h2 in range(2):
                                    for t1 in range(2):
                                        st = _sgn1d(dt, t2, t1)
                                        for h1 in range(2):
                                            sh = _sgn1d(hp, h2, h1)
                                            m = (w1 * 64 + w2 * 32
                                                 + (2 * t2 + h2) * 8
                                                 + yp * 4 + (2 * t1 + h1))
                                            w[k, m] = 2.0 * st * sh * sw2 * sw1
    return w.astype(_BF16_NP)


def _build_nc():
    nc = bacc.Bacc(
        "TRN2", target_bir_lowering=False, debug=False, num_devices=N_CORES
    )
    # SBUF image per t: [128, 5808] bf16; k = (hh, dt, p2, p1), f = (ci,c,x)
    x = nc.dram_tensor(
        "x", [T_DEV, 128, NCH * CX], _BF16, kind="ExternalInput"
    ).ap()
    w8 = nc.dram_tensor("w8", [128, 128], _BF16, kind="ExternalInput").ap()
    # final subbands, uint8, padded per store group (group 2 uses 1584B of
    # its 2112B slot); q = v*128 + 128 via round-to-nearest
    out = nc.dram_tensor(
        "out", [T_DEV, 3, 128, 4 * CX], _U8, kind="ExternalOutput"
    ).ap()

    with tile.TileContext(nc) as tc:
        with (
            tc.tile_pool(name="signs", bufs=1) as sgp,
            tc.tile_pool(name="rv0", bufs=1) as r0p,
            tc.tile_pool(name="rhs", bufs=5) as rhp,
            tc.tile_pool(name="outp", bufs=6) as otp,
            tc.tile_pool(name="psum", bufs=8, space="PSUM") as psp,
        ):
            # ALL early DMAs on sync, in priority order: single-queue gen
            # order = serialized-transfer-pool order, so w8 (tiny) and the
            # t=0 halves land first and mm0 starts right after the queue
            # preamble clears (~7us) + ~4us of transfers
            wt = sgp.tile([128, 128], _BF16, tag="w8")
            nc.sync.dma_start(out=wt, in_=w8)
            rv0 = r0p.tile([128, NCH, C, X], _BF16, tag="rv0")
            rv0f = rv0.rearrange("k g c x -> k (g c x)")
            HCX = NCH * CX // 2
            nc.sync.dma_start(out=rv0f[:, :HCX], in_=x[0, :, :HCX])
            nc.sync.dma_start(out=rv0f[:, HCX:], in_=x[0, :, HCX:])

            def issue_load(t):
                # two half-t DMAs (5808B descriptors): finer arrival
                # granularity so the first chunks' matmuls start ~1.8us
                # before the tail of the slab lands
                rv = rhp.tile([128, NCH, C, X], _BF16, tag="rhs")
                fl = rv.rearrange("k g c x -> k (g c x)")
                nc.sync.dma_start(out=fl[:, :HCX], in_=x[t, :, :HCX])
                nc.sync.dma_start(out=fl[:, HCX:], in_=x[t, :, HCX:])
                return rv

            rvs = {0: rv0}
            for t in range(1, min(4, T_DEV)):
                rvs[t] = issue_load(t)

            for t in range(T_DEV):
                if t + 4 < T_DEV:
                    rvs[t + 4] = issue_load(t + 4)
                rv = rvs.pop(t)
                for gi, (g0, gn) in enumerate(GROUPS):
                    ot = otp.tile([128, 4, C, X], _U8, tag="ot")
                    otf = ot.rearrange("k g c x -> k (g c) x")
                    # ALL copies of a group on ONE engine (alternating per
                    # group): copies into a shared ot tile from different
                    # engines get WAW cross-serialized by tile tracking,
                    # each with a ~150ns sem round-trip, which stalls the
                    # matmuls through the 16-slot PSUM WAR window
                    sc_grp = (t * 3 + gi) % 2 == 0
                    nmm = gn * C
                    for j in range((nmm + 1) // 2):
                        nsl = min(2, nmm - 2 * j)
                        ps = psp.tile([128, 2, 256], _F32, tag="ps")
                        for sl in range(nsl):
                            mi = 2 * j + sl
                            g, c = divmod(mi, C)
                            nc.tensor.matmul(
                                ps[:, sl, :X], wt, rv[:, g0 + g, c],
                                start=True, stop=True,
                            )
                        dst = otf[:, 2 * j : 2 * j + nsl, :]
                        src = ps[:, :nsl, :X]
                        if sc_grp:
                            nc.scalar.activation(
                                out=dst, in_=src,
                                func=mybir.ActivationFunctionType.Copy,
                                bias=128.5, scale=1.0,
                            )
                        else:
                            nc.vector.tensor_scalar_add(
                                out=dst, in0=src, scalar1=128.5
                            )
                    nc.gpsimd.dma_start(
                        out=out[t, gi, :, : gn * CX],
                        in_=ot[:, :gn].rearrange("k g c x -> k (g c x)"),
                    )

    nc.compile()
    return nc


_NC_CACHE = None


def _prep_inputs(hs):
    """Shard along H, quantize frames 1..32 to bf16, pack the SBUF image."""
    w8 = _build_w()
    in_maps = []
    for k in range(N_CORES):
        xk = hs[0, :, 1:, k * H_SH : (k + 1) * H_SH, :]  # [C, 32, 88, 704]
        q = np.ascontiguousarray(xk).astype(_BF16_NP)
        # w = 4*x'' + 2*p2 + p1; h = chunk*8 + hh
        r = q.reshape(C, T_DEV, 4, NCH, 8, X, 2, 2)  # c,t,dt,ci,hh,x'',p2,p1
        # -> [t, hh, dt, p2, p1, ci, c, x'']
        r = r.transpose(1, 4, 2, 6, 7, 3, 0, 5)
        x5 = np.ascontiguousarray(r).reshape(T_DEV, 128, NCH * CX)
        in_maps.append({"x": x5, "w8": w8})
    return in_maps


def _host_t0(hs, ov):
    """Fill the t=0 output block: 2D 2-level Haar of frame 0, exact f32.

    ov[th2, w2, th1, w1, c, t, y, x]; at t=0 only T2=T1=0 survive
    (th2 = h2 < 2, th1 = h1 < 2), value = (1/16) * sum of signed 4x4.
    """
    f0 = np.ascontiguousarray(hs[0, :, 0]).astype(np.float32)  # [C,704,704]
    A = np.zeros((4, 4), np.float32)
    for b in range(4):
        for p in range(4):
            A[b, p] = _sgn1d(p, b >> 1, b & 1)
    f0r = f0.reshape(C, X, 4, X, 4)
    t0 = np.einsum("ah,bw,cyhxw->abcyx", A, A, f0r) * (1.0 / 16.0)
    ov[:, :, :, :, :, 0] = 0.0
    for a in range(4):
        h2, h1 = a >> 1, a & 1
        for b in range(4):
            w2, w1 = b >> 1, b & 1
            ov[h2, w2, h1, w1, :, 0] = t0[a, b]


def kernel(hidden_states: np.ndarray) -> np.ndarray:
    global _NC_CACHE
    if _NC_CACHE is None:
        _NC_CACHE = _build_nc()
    nc = _NC_CACHE

    hs = np.asarray(hidden_states, dtype=np.float32)
    assert hs.shape == (1, C, T_IN, H_IN, W_IN), hs.shape
    in_maps = _prep_inputs(hs)

    res = run_bass_kernel_spmd(nc, in_maps, core_ids=list(range(N_CORES)))

    out = np.empty((1, 64 * C, T_IN // 4 + 1, H_IN // 4, X), dtype=np.float32)
    ov = out[0].reshape(4, 2, 4, 2, C, T_IN // 4 + 1, H_IN // 4, X)
    _host_t0(hs, ov)
    for k in range(N_CORES):
        o = np.asarray(res.results[k]["out"])  # [8, 3, 128, 2112]
        full = np.empty((T_DEV, NCH, 128, CX), dtype=np.uint8)
        for gi, (g0, gn) in enumerate(GROUPS):
            full[:, g0 : g0 + gn] = (
                o[:, gi, :, : gn * CX]
                .reshape(T_DEV, 128, gn, CX).transpose(0, 2, 1, 3)
            )
        v = (full.astype(np.float32) - 128.0) * (1.0 / 128.0)
        # [t, ci, m, c, x]; m = (w1, w2, th2, y', th1)
        v = v.reshape(T_DEV, NCH, 2, 2, 4, 2, 4, C, X)
        # -> ov[th2, w2, th1, w1, c, t(1..8), y=(ci,y'), x]
        a = v.transpose(4, 3, 6, 2, 7, 0, 1, 5, 8)  # th2,w2,th1,w1,c,t,ci,y',x
        ov[:, :, :, :, :, 1:, k * Y_SH : (k + 1) * Y_SH, :] = a.reshape(
            4, 2, 4, 2, C, T_DEV, Y_SH, X
        )
    return out


# revision 29
# speedup vs baseline: 1.0682x; 1.0682x over previous
"""Trainium2 Bass kernel for nn_CosmosPatcher3d.

Computes the Cosmos 3D Haar wavelet patcher: input [1,3,33,704,704] fp32,
temporal causal pad (first frame repeated 4x -> 36 frames), then two full
3D Haar DWT levels. Equivalent to a separable +-1 Hadamard transform over
4x4x4 blocks scaled by 1/64, producing [1,192,9,176,176] fp32 with channel
layout ch = 48*TH2 + 24*W2 + 6*TH1 + 3*W1 + c (TH = 2T+H).

v4 strategy (8 NeuronCores, shard along H: 704 = 8*88):
- t-block 0 is the repeated first frame: its T-highpass bands are exactly
  zero and the rest is a 2D transform of frame 0 -> computed on HOST in
  f32. The device handles t-blocks 1..8 (frames 1..32) only.
- The ENTIRE 3-level butterfly runs inside ONE plain bf16 matmul per
  (chunk, c): both W parities (p1, p2) are moved into the contraction
  dim: partition k = (hh in 8, dt in 4, p2, p1) = 128, output rows
  m = (w1, w2, th2, y', th1) = 128. Each of the 64 block inputs maps to
  exactly one k; cols = x'' (176). PE cost = 1 col/cycle at the 1.2GHz
  mid p-state = 4.84us/t. Ldweights between back-to-back same-weight
  matmuls are free; the 2.4GHz boost resets on sub-us idle gaps and is
  unreachable in steady state, so the schedule assumes 1.2GHz.
- Input bf16 (absmax rel err 1.8e-3), output = FINAL subband values in
  uint8: the x128 scale is folded into the sign matrix (entries +-2,
  exact) and the copies add +128.5 so the truncating f32->uint8 store
  becomes round-to-nearest (q = v*128 + 128, err 3.9e-3; host decodes
  (q-128)/128). Total err ~7e-3 << 2e-2.
- Per-core DMA 17.84MB (11.89 bf16 in + 5.95 uint8 out). Loads: ONE
  whole-t [128, 11616B] HWDGE DMA on sync (11616B descriptors ~427GB/s
  on the serialized 16-engine pool). Stores: one [128, <=2112B] SWDGE
  DMA per 4-chunk group on GPSIMD -- stores must stay OFF the scalar/
  vector queues: a DMA instruction between copies stalls the copy stream
  past the 16-slot PSUM WAR window and the matmuls then wait on copy
  semaphores. Copies alternate scalar/vector (round-robin per matmul).
"""

import ml_dtypes
import numpy as np

import concourse.bacc as bacc
import concourse.mybir as mybir
import concourse.tile as tile
from concourse.bass_utils import run_bass_kernel_spmd

N_CORES = 8
C = 3              # input channels
T_IN = 33          # input frames
H_IN = 704         # input height (global)
W_IN = 704         # input width
H_SH = H_IN // N_CORES       # 88 input rows per core
T_DEV = 8          # device t-blocks (1..8); t=0 on host
X = W_IN // 4      # 176 output cols
Y_SH = H_SH // 4   # 22 output rows per core
NCH = 11           # chunks of nh=8 h-rows per t
GROUPS = [(0, 4), (4, 4), (8, 3)]  # (first chunk, n chunks) per store group
CX = C * X         # 528 free elements per chunk row

_F32 = mybir.dt.float32
_BF16 = mybir.dt.bfloat16
_U8 = mybir.dt.uint8
_BF16_NP = ml_dtypes.bfloat16


def _sgn1d(pos, b2, b1):
    """Composite 2-level Haar sign for position pos in 0..3 (+-1)."""
    s1 = 1.0 if b1 == 0 else (1.0 - 2.0 * (pos % 2))
    s2 = 1.0 if b2 == 0 else (1.0 - 2.0 * (pos // 2))
    return s1 * s2


def _build_w():
    """[128, 128] bf16 sign matrix, all 3 butterfly levels + x2 scale.

    k = hh*16 + dt*4 + p2*2 + p1 (hh in 0..8),
    m = w1*64 + w2*32 + th2*8 + y'*4 + th1 (y' = hh//4).
    Entries +-2 = 128/64: folds the global 1/64 and the x128 uint8 scale.
    """
    w = np.zeros((128, 128), dtype=np.float32)
    for hh in range(8):
        yp, hp = hh // 4, hh % 4
        for dt in range(4):
            for p2 in range(2):
                for p1 in range(2):
                    k = hh * 16 + dt * 4 + p2 * 2 + p1
                    for w1 in range(2):
                        sw1 = 1.0 if w1 == 0 else (1.0 - 2.0 * p1)
                        for w2 in range(2):
                            sw2 = 1.0 if w2 == 0 else (1.0 - 2.0 * p2)
                            for t2 in range(2):
                                for h2 in range(2):
                                    for t1 in range(2):
                                        st = _sgn1d(dt, t2, t1)
                                        for h1 in range(2):
                                            sh = _sgn1d(hp, h2, h1)
                                            m = (w1 * 64 + w2 * 32
                                                 + (2 * t2 + h2) * 8
                                                 + yp * 4 + (2 * t1 + h1))
                                            w[k, m] = 2.0 * st * sh * sw2 * sw1
    return w.astype(_BF16_NP)


def _build_nc():
    nc = bacc.Bacc(
        "TRN2", target_bir_lowering=False, debug=False, num_devices=N_CORES
    )
    # SBUF image per t: [128, 5808] bf16; k = (hh, dt, p2, p1), f = (ci,c,x)
    x = nc.dram_tensor(
        "x", [T_DEV, 128, NCH * CX], _BF16, kind="ExternalInput"
    ).ap()
    w8 = nc.dram_tensor("w8", [128, 128], _BF16, kind="ExternalInput").ap()
    # final subbands, uint8, padded per store group (group 2 uses 1584B of
    # its 2112B slot); q = v*128 + 128 via round-to-nearest
    out = nc.dram_tensor(
        "out", [T_DEV, 3, 128, 4 * CX], _U8, kind="ExternalOutput"
    ).ap()

    with tile.TileContext(nc) as tc:
        with (
            tc.tile_pool(name="signs", bufs=1) as sgp,
            tc.tile_pool(name="rv0", bufs=1) as r0p,
            tc.tile_pool(name="rhs", bufs=5) as rhp,
            tc.tile_pool(name="outp", bufs=6) as otp,
            tc.tile_pool(name="psum", bufs=8, space="PSUM") as psp,
        ):
            # w8 + the t=0 slab issued on scalar (its own queue, so their
            # DGE gens race ahead of the sync prefetch burst and the t=0
            # data lands early on the serialized transfer pool)
            wt = sgp.tile([128, 128], _BF16, tag="w8")
            nc.scalar.dma_start(out=wt, in_=w8)
            rv0 = r0p.tile([128, NCH, C, X], _BF16, tag="rv0")
            rv0f = rv0.rearrange("k g c x -> k (g c x)")
            HCX = NCH * CX // 2
            nc.scalar.dma_start(out=rv0f[:, :HCX], in_=x[0, :, :HCX])
            nc.scalar.dma_start(out=rv0f[:, HCX:], in_=x[0, :, HCX:])

            def issue_load(t):
                # two half-t DMAs (5808B descriptors): finer arrival
                # granularity so the first chunks' matmuls start ~1.8us
                # before the tail of the slab lands
                rv = rhp.tile([128, NCH, C, X], _BF16, tag="rhs")
                fl = rv.rearrange("k g c x -> k (g c x)")
                nc.sync.dma_start(out=fl[:, :HCX], in_=x[t, :, :HCX])
                nc.sync.dma_start(out=fl[:, HCX:], in_=x[t, :, HCX:])
                return rv

            rvs = {0: rv0}
            for t in range(1, min(4, T_DEV)):
                rvs[t] = issue_load(t)

            for t in range(T_DEV):
                if t + 4 < T_DEV:
                    rvs[t + 4] = issue_load(t + 4)
                rv = rvs.pop(t)
                for gi, (g0, gn) in enumerate(GROUPS):
                    ot = otp.tile([128, 4, C, X], _U8, tag="ot")
                    otf = ot.rearrange("k g c x -> k (g c) x")
                    # ALL copies of a group on ONE engine (alternating per
                    # group): copies into a shared ot tile from different
                    # engines get WAW cross-serialized by tile tracking,
                    # each with a ~150ns sem round-trip, which stalls the
                    # matmuls through the 16-slot PSUM WAR window
                    sc_grp = (t * 3 + gi) % 2 == 0
                    nmm = gn * C
                    for j in range((nmm + 1) // 2):
                        nsl = min(2, nmm - 2 * j)
                        ps = psp.tile([128, 2, 256], _F32, tag="ps")
                        for sl in range(nsl):
                            mi = 2 * j + sl
                            g, c = divmod(mi, C)
                            nc.tensor.matmul(
                                ps[:, sl, :X], wt, rv[:, g0 + g, c],
                                start=True, stop=True,
                            )
                        dst = otf[:, 2 * j : 2 * j + nsl, :]
                        src = ps[:, :nsl, :X]
                        if sc_grp:
                            nc.scalar.activation(
                                out=dst, in_=src,
                                func=mybir.ActivationFunctionType.Copy,
                                bias=128.5, scale=1.0,
                            )
                        else:
                            nc.vector.tensor_scalar_add(
                                out=dst, in0=src, scalar1=128.5
                            )
                    nc.gpsimd.dma_start(
                        out=out[t, gi, :, : gn * CX],
                        in_=ot[:, :gn].rearrange("k g c x -> k (g c x)"),
                    )

    nc.compile()
    return nc


_NC_CACHE = None


def _prep_inputs(hs):
    """Shard along H, quantize frames 1..32 to bf16, pack the SBUF image."""
    w8 = _build_w()
    in_maps = []
    for k in range(N_CORES):
        xk = hs[0, :, 1:, k * H_SH : (k + 1) * H_SH, :]  # [C, 32, 88, 704]
        q = np.ascontiguousarray(xk).astype(_BF16_NP)
        # w = 4*x'' + 2*p2 + p1; h = chunk*8 + hh
        r = q.reshape(C, T_DEV, 4, NCH, 8, X, 2, 2)  # c,t,dt,ci,hh,x'',p2,p1
        # -> [t, hh, dt, p2, p1, ci, c, x'']
        r = r.transpose(1, 4, 2, 6, 7, 3, 0, 5)
        x5 = np.ascontiguousarray(r).reshape(T_DEV, 128, NCH * CX)
        in_maps.append({"x": x5, "w8": w8})
    return in_maps


def _host_t0(hs, ov):
    """Fill the t=0 output block: 2D 2-level Haar of frame 0, exact f32.

    ov[th2, w2, th1, w1, c, t, y, x]; at t=0 only T2=T1=0 survive
    (th2 = h2 < 2, th1 = h1 < 2), value = (1/16) * sum of signed 4x4.
    """
    f0 = np.ascontiguousarray(hs[0, :, 0]).astype(np.float32)  # [C,704,704]
    A = np.zeros((4, 4), np.float32)
    for b in range(4):
        for p in range(4):
            A[b, p] = _sgn1d(p, b >> 1, b & 1)
    f0r = f0.reshape(C, X, 4, X, 4)
    t0 = np.einsum("ah,bw,cyhxw->abcyx", A, A, f0r) * (1.0 / 16.0)
    ov[:, :, :, :, :, 0] = 0.0
    for a in range(4):
        h2, h1 = a >> 1, a & 1
        for b in range(4):
            w2, w1 = b >> 1, b & 1
            ov[h2, w2, h1, w1, :, 0] = t0[a, b]


def kernel(hidden_states: np.ndarray) -> np.ndarray:
    global _NC_CACHE
    if _NC_CACHE is None:
        _NC_CACHE = _build_nc()
    nc = _NC_CACHE

    hs = np.asarray(hidden_states, dtype=np.float32)
    assert hs.shape == (1, C, T_IN, H_IN, W_IN), hs.shape
    in_maps = _prep_inputs(hs)

    res = run_bass_kernel_spmd(nc, in_maps, core_ids=list(range(N_CORES)))

    out = np.empty((1, 64 * C, T_IN // 4 + 1, H_IN // 4, X), dtype=np.float32)
    ov = out[0].reshape(4, 2, 4, 2, C, T_IN // 4 + 1, H_IN // 4, X)
    _host_t0(hs, ov)
    for k in range(N_CORES):
        o = np.asarray(res.results[k]["out"])  # [8, 3, 128, 2112]
        full = np.empty((T_DEV, NCH, 128, CX), dtype=np.uint8)
        for gi, (g0, gn) in enumerate(GROUPS):
            full[:, g0 : g0 + gn] = (
                o[:, gi, :, : gn * CX]
                .reshape(T_DEV, 128, gn, CX).transpose(0, 2, 1, 3)
            )
        v = (full.astype(np.float32) - 128.0) * (1.0 / 128.0)
        # [t, ci, m, c, x]; m = (w1, w2, th2, y', th1)
        v = v.reshape(T_DEV, NCH, 2, 2, 4, 2, 4, C, X)
        # -> ov[th2, w2, th1, w1, c, t(1..8), y=(ci,y'), x]
        a = v.transpose(4, 3, 6, 2, 7, 0, 1, 5, 8)  # th2,w2,th1,w1,c,t,ci,y',x
        ov[:, :, :, :, :, 1:, k * Y_SH : (k + 1) * Y_SH, :] = a.reshape(
            4, 2, 4, 2, C, T_DEV, Y_SH, X
        )
    return out


# revision 30
# speedup vs baseline: 1.0728x; 1.0043x over previous
"""Trainium2 Bass kernel for nn_CosmosPatcher3d.

Computes the Cosmos 3D Haar wavelet patcher: input [1,3,33,704,704] fp32,
temporal causal pad (first frame repeated 4x -> 36 frames), then two full
3D Haar DWT levels. Equivalent to a separable +-1 Hadamard transform over
4x4x4 blocks scaled by 1/64, producing [1,192,9,176,176] fp32 with channel
layout ch = 48*TH2 + 24*W2 + 6*TH1 + 3*W1 + c (TH = 2T+H).

v4 strategy (8 NeuronCores, shard along H: 704 = 8*88):
- t-block 0 is the repeated first frame: its T-highpass bands are exactly
  zero and the rest is a 2D transform of frame 0 -> computed on HOST in
  f32. The device handles t-blocks 1..8 (frames 1..32) only.
- The ENTIRE 3-level butterfly runs inside ONE plain bf16 matmul per
  (chunk, c): both W parities (p1, p2) are moved into the contraction
  dim: partition k = (hh in 8, dt in 4, p2, p1) = 128, output rows
  m = (w1, w2, th2, y', th1) = 128. Each of the 64 block inputs maps to
  exactly one k; cols = x'' (176). PE cost = 1 col/cycle at the 1.2GHz
  mid p-state = 4.84us/t. Ldweights between back-to-back same-weight
  matmuls are free; the 2.4GHz boost resets on sub-us idle gaps and is
  unreachable in steady state, so the schedule assumes 1.2GHz.
- Input bf16 (absmax rel err 1.8e-3), output = FINAL subband values in
  uint8: the x128 scale is folded into the sign matrix (entries +-2,
  exact) and the copies add +128.5 so the truncating f32->uint8 store
  becomes round-to-nearest (q = v*128 + 128, err 3.9e-3; host decodes
  (q-128)/128). Total err ~7e-3 << 2e-2.
- Per-core DMA 17.84MB (11.89 bf16 in + 5.95 uint8 out). Loads: ONE
  whole-t [128, 11616B] HWDGE DMA on sync (11616B descriptors ~427GB/s
  on the serialized 16-engine pool). Stores: one [128, <=2112B] SWDGE
  DMA per 4-chunk group on GPSIMD -- stores must stay OFF the scalar/
  vector queues: a DMA instruction between copies stalls the copy stream
  past the 16-slot PSUM WAR window and the matmuls then wait on copy
  semaphores. Copies alternate scalar/vector (round-robin per matmul).
"""

import ml_dtypes
import numpy as np

import concourse.bacc as bacc
import concourse.mybir as mybir
import concourse.tile as tile
from concourse.bass_utils import run_bass_kernel_spmd

N_CORES = 8
C = 3              # input channels
T_IN = 33          # input frames
H_IN = 704         # input height (global)
W_IN = 704         # input width
H_SH = H_IN // N_CORES       # 88 input rows per core
T_DEV = 8          # device t-blocks (1..8); t=0 on host
X = W_IN // 4      # 176 output cols
Y_SH = H_SH // 4   # 22 output rows per core
NCH = 11           # chunks of nh=8 h-rows per t
GROUPS = [(0, 4), (4, 4), (8, 3)]  # (first chunk, n chunks) per store group
CX = C * X         # 528 free elements per chunk row

_F32 = mybir.dt.float32
_BF16 = mybir.dt.bfloat16
_U8 = mybir.dt.uint8
_BF16_NP = ml_dtypes.bfloat16


def _sgn1d(pos, b2, b1):
    """Composite 2-level Haar sign for position pos in 0..3 (+-1)."""
    s1 = 1.0 if b1 == 0 else (1.0 - 2.0 * (pos % 2))
    s2 = 1.0 if b2 == 0 else (1.0 - 2.0 * (pos // 2))
    return s1 * s2


def _build_w():
    """[128, 128] bf16 sign matrix, all 3 butterfly levels + x2 scale.

    k = hh*16 + dt*4 + p2*2 + p1 (hh in 0..8),
    m = w1*64 + w2*32 + th2*8 + y'*4 + th1 (y' = hh//4).
    Entries +-2 = 128/64: folds the global 1/64 and the x128 uint8 scale.
    """
    w = np.zeros((128, 128), dtype=np.float32)
    for hh in range(8):
        yp, hp = hh // 4, hh % 4
        for dt in range(4):
            for p2 in range(2):
                for p1 in range(2):
                    k = hh * 16 + dt * 4 + p2 * 2 + p1
                    for w1 in range(2):
                        sw1 = 1.0 if w1 == 0 else (1.0 - 2.0 * p1)
                        for w2 in range(2):
                            sw2 = 1.0 if w2 == 0 else (1.0 - 2.0 * p2)
                            for t2 in range(2):
                                for h2 in range(2):
                                    for t1 in range(2):
                                        st = _sgn1d(dt, t2, t1)
                                        for h1 in range(2):
                                            sh = _sgn1d(hp, h2, h1)
                                            m = (w1 * 64 + w2 * 32
                                                 + (2 * t2 + h2) * 8
                                                 + yp * 4 + (2 * t1 + h1))
                                            w[k, m] = 2.0 * st * sh * sw2 * sw1
    return w.astype(_BF16_NP)


def _build_nc():
    nc = bacc.Bacc(
        "TRN2", target_bir_lowering=False, debug=False, num_devices=N_CORES
    )
    # SBUF image per t: [128, 5808] bf16; k = (hh, dt, p2, p1), f = (ci,c,x)
    x = nc.dram_tensor(
        "x", [T_DEV, 128, NCH * CX], _BF16, kind="ExternalInput"
    ).ap()
    w8 = nc.dram_tensor("w8", [128, 128], _BF16, kind="ExternalInput").ap()
    # final subbands, uint8, padded per store group (group 2 uses 1584B of
    # its 2112B slot); q = v*128 + 128 via round-to-nearest
    out = nc.dram_tensor(
        "out", [T_DEV, 3, 128, 4 * CX], _U8, kind="ExternalOutput"
    ).ap()

    with tile.TileContext(nc) as tc:
        with (
            tc.tile_pool(name="signs", bufs=1) as sgp,
            tc.tile_pool(name="rv0", bufs=1) as r0p,
            tc.tile_pool(name="rhs", bufs=6) as rhp,
            tc.tile_pool(name="outp", bufs=6) as otp,
            tc.tile_pool(name="psum", bufs=8, space="PSUM") as psp,
        ):
            # w8 + the t=0 slab issued on scalar (its own queue, so their
            # DGE gens race ahead of the sync prefetch burst and the t=0
            # data lands early on the serialized transfer pool)
            wt = sgp.tile([128, 128], _BF16, tag="w8")
            nc.scalar.dma_start(out=wt, in_=w8)
            rv0 = r0p.tile([128, NCH, C, X], _BF16, tag="rv0")
            rv0f = rv0.rearrange("k g c x -> k (g c x)")
            HCX = NCH * CX // 2
            nc.scalar.dma_start(out=rv0f[:, :HCX], in_=x[0, :, :HCX])
            nc.scalar.dma_start(out=rv0f[:, HCX:], in_=x[0, :, HCX:])

            def issue_load(t):
                # two half-t DMAs (5808B descriptors): finer arrival
                # granularity so the first chunks' matmuls start ~1.8us
                # before the tail of the slab lands
                rv = rhp.tile([128, NCH, C, X], _BF16, tag="rhs")
                fl = rv.rearrange("k g c x -> k (g c x)")
                nc.sync.dma_start(out=fl[:, :HCX], in_=x[t, :, :HCX])
                nc.sync.dma_start(out=fl[:, HCX:], in_=x[t, :, HCX:])
                return rv

            rvs = {0: rv0}
            for t in range(1, min(5, T_DEV)):
                rvs[t] = issue_load(t)

            for t in range(T_DEV):
                if t + 5 < T_DEV:
                    rvs[t + 5] = issue_load(t + 5)
                rv = rvs.pop(t)
                for gi, (g0, gn) in enumerate(GROUPS):
                    ot = otp.tile([128, 4, C, X], _U8, tag="ot")
                    otf = ot.rearrange("k g c x -> k (g c) x")
                    # ALL copies of a group on ONE engine (alternating per
                    # group): copies into a shared ot tile from different
                    # engines get WAW cross-serialized by tile tracking,
                    # each with a ~150ns sem round-trip, which stalls the
                    # matmuls through the 16-slot PSUM WAR window
                    sc_grp = (t * 3 + gi) % 2 == 0
                    nmm = gn * C
                    for j in range((nmm + 1) // 2):
                        nsl = min(2, nmm - 2 * j)
                        ps = psp.tile([128, 2, 256], _F32, tag="ps")
                        for sl in range(nsl):
                            mi = 2 * j + sl
                            g, c = divmod(mi, C)
                            nc.tensor.matmul(
                                ps[:, sl, :X], wt, rv[:, g0 + g, c],
                                start=True, stop=True,
                            )
                        dst = otf[:, 2 * j : 2 * j + nsl, :]
                        src = ps[:, :nsl, :X]
                        if sc_grp:
                            nc.scalar.activation(
                                out=dst, in_=src,
                                func=mybir.ActivationFunctionType.Copy,
                                bias=128.5, scale=1.0,
                            )
                        else:
                            nc.vector.tensor_scalar_add(
                                out=dst, in0=src, scalar1=128.5
                            )
                    nc.gpsimd.dma_start(
                        out=out[t, gi, :, : gn * CX],
                        in_=ot[:, :gn].rearrange("k g c x -> k (g c x)"),
                    )

    nc.compile()
    return nc


_NC_CACHE = None


def _prep_inputs(hs):
    """Shard along H, quantize frames 1..32 to bf16, pack the SBUF image."""
    w8 = _build_w()
    in_maps = []
    for k in range(N_CORES):
        xk = hs[0, :, 1:, k * H_SH : (k + 1) * H_SH, :]  # [C, 32, 88, 704]
        q = np.ascontiguousarray(xk).astype(_BF16_NP)
        # w = 4*x'' + 2*p2 + p1; h = chunk*8 + hh
        r = q.reshape(C, T_DEV, 4, NCH, 8, X, 2, 2)  # c,t,dt,ci,hh,x'',p2,p1
        # -> [t, hh, dt, p2, p1, ci, c, x'']
        r = r.transpose(1, 4, 2, 6, 7, 3, 0, 5)
        x5 = np.ascontiguousarray(r).reshape(T_DEV, 128, NCH * CX)
        in_maps.append({"x": x5, "w8": w8})
    return in_maps


def _host_t0(hs, ov):
    """Fill the t=0 output block: 2D 2-level Haar of frame 0, exact f32.

    ov[th2, w2, th1, w1, c, t, y, x]; at t=0 only T2=T1=0 survive
    (th2 = h2 < 2, th1 = h1 < 2), value = (1/16) * sum of signed 4x4.
    """
    f0 = np.ascontiguousarray(hs[0, :, 0]).astype(np.float32)  # [C,704,704]
    A = np.zeros((4, 4), np.float32)
    for b in range(4):
        for p in range(4):
            A[b, p] = _sgn1d(p, b >> 1, b & 1)
    f0r = f0.reshape(C, X, 4, X, 4)
    t0 = np.einsum("ah,bw,cyhxw->abcyx", A, A, f0r) * (1.0 / 16.0)
    ov[:, :, :, :, :, 0] = 0.0
    for a in range(4):
        h2, h1 = a >> 1, a & 1
        for b in range(4):
            w2, w1 = b >> 1, b & 1
            ov[h2, w2, h1, w1, :, 0] = t0[a, b]


def kernel(hidden_states: np.ndarray) -> np.ndarray:
    global _NC_CACHE
    if _NC_CACHE is None:
        _NC_CACHE = _build_nc()
    nc = _NC_CACHE

    hs = np.asarray(hidden_states, dtype=np.float32)
    assert hs.shape == (1, C, T_IN, H_IN, W_IN), hs.shape
    in_maps = _prep_inputs(hs)

    res = run_bass_kernel_spmd(nc, in_maps, core_ids=list(range(N_CORES)))

    out = np.empty((1, 64 * C, T_IN // 4 + 1, H_IN // 4, X), dtype=np.float32)
    ov = out[0].reshape(4, 2, 4, 2, C, T_IN // 4 + 1, H_IN // 4, X)
    _host_t0(hs, ov)
    for k in range(N_CORES):
        o = np.asarray(res.results[k]["out"])  # [8, 3, 128, 2112]
        full = np.empty((T_DEV, NCH, 128, CX), dtype=np.uint8)
        for gi, (g0, gn) in enumerate(GROUPS):
            full[:, g0 : g0 + gn] = (
                o[:, gi, :, : gn * CX]
                .reshape(T_DEV, 128, gn, CX).transpose(0, 2, 1, 3)
            )
        v = (full.astype(np.float32) - 128.0) * (1.0 / 128.0)
        # [t, ci, m, c, x]; m = (w1, w2, th2, y', th1)
        v = v.reshape(T_DEV, NCH, 2, 2, 4, 2, 4, C, X)
        # -> ov[th2, w2, th1, w1, c, t(1..8), y=(ci,y'), x]
        a = v.transpose(4, 3, 6, 2, 7, 0, 1, 5, 8)  # th2,w2,th1,w1,c,t,ci,y',x
        ov[:, :, :, :, :, 1:, k * Y_SH : (k + 1) * Y_SH, :] = a.reshape(
            4, 2, 4, 2, C, T_DEV, Y_SH, X
        )
    return out
